# revision 1
# baseline (speedup 1.0000x reference)
"""Trainium2 Bass kernel: batched polynomial + Fourier-series point evaluator.

Math: for each point n and each of B=4 times t_b:
    y_poly[b, n]    = sum_{i<4}  poly[n, i] * t_b^i
    y_fourier[b, n] = sum_{k<18} fa[n, k]*cos(w_k t_b) + fb[n, k]*sin(w_k t_b)
(with Fourier bands gated by model_stage).

Because B=4 is tiny, both outputs are one linear map applied to the 40
per-point coefficients:  Y[:, n] = Basis.T @ W[n, :]  with Basis [40, 8]
computed on host (the transcendentals depend only on the 4 scalar times).
The device kernel is a pure streaming matmul over the coefficient tables.

Per-core layout (points sharded 8 ways, ~2^18 points/core, padded to
NP = 3*C so the contraction dim packs GROUPS=3 point-groups of 40 coeffs
= K=120):
  - host packs coefficients as fp16 [120, C]; each matmul column carries
    3 points, so one N=512 matmul evaluates 1536 points.
  - 4 matmuls per PSUM bank at tile_position (0, 32j) run concurrently on
    disjoint 32-column strips of the PE array (same [120, 32] stationary
    basis), filling a [128, 512] bank with 6144 points' outputs.
  - PSUM -> SBUF copies cast to fp16; row 32j+8g+jj then holds output jj
    (0-3 poly batch, 4-7 fourier batch) of group g on a contiguous point
    run, so output DMAs are plain 2D slices.
"""

import json

import numpy as np

import concourse.bass as bass
import concourse.mybir as mybir
import concourse.tile as tile
from concourse.bass_utils import run_bass_kernel_spmd

# Problem constants (hardcoded per harness contract).
B = 4
N_POINTS = 128 ** 3            # 2097152
N_CORES = 8
NC = N_POINTS // N_CORES       # 262144 real points per core
KH = 18                        # harmonics
NCOEF = 40                     # 4 poly + 18 cos + 18 sin

GROUPS = 3                     # point-groups stacked in contraction dim (K=120)
JT = 4                         # concurrent col-strip matmuls per PSUM bank
MM_N = 512                     # matmul moving free size (one PSUM bank of fp32)
SPANS = 2
U = 22                         # PSUM bank fills per span
BANK_COLS = JT * MM_N          # 2048 table columns per bank fill
C = SPANS * U * BANK_COLS      # 90112 table columns per core
NP = GROUPS * C                # 270336 padded points per core

_CACHED_NC = None
LAST_RESULTS = None            # BassKernelResults of the most recent run


def _build_module():
    nc = bass.Bass()
    dt = mybir.dt

    # Tiled layouts: every DMA moves one fully contiguous DRAM block.
    table = nc.dram_tensor(
        "table", [C // (2 * BANK_COLS), GROUPS * NCOEF, 2 * BANK_COLS],
        dt.float16, kind="ExternalInput")
    basis = nc.dram_tensor("basis", [GROUPS * NCOEF, 32], dt.float16,
                           kind="ExternalInput")
    out_t = nc.dram_tensor(
        "out_t", [SPANS, 2, JT, GROUPS, 8, U * MM_N // 2],
        dt.float16, kind="ExternalOutput")

    with tile.TileContext(nc) as tc:
        with (
            tc.tile_pool(name="const", bufs=1) as cpool,
            tc.tile_pool(name="inp", bufs=8) as ipool,
            tc.tile_pool(name="psum", bufs=8, space="PSUM") as ppool,
            tc.tile_pool(name="outp", bufs=3) as opool,
        ):
            basis_sb = cpool.tile([GROUPS * NCOEF, 32], dt.float16)
            nc.sync.dma_start(basis_sb[:, :], basis[:, :])

            HU = U // 2
            for span in range(SPANS):
                out_tile = opool.tile([128, U * MM_N], dt.float16)
                for it in range(HU):
                    # One in-DMA covers two bank fills; alternate HWDGE (SP)
                    # and SWDGE (Pool) so dispatch overheads run in parallel.
                    in_tile = ipool.tile(
                        [GROUPS * NCOEF, 2 * BANK_COLS], dt.float16
                    )
                    eng = nc.sync if it % 2 == 0 else nc.gpsimd
                    eng.dma_start(in_tile[:, :], table[span * HU + it])
                    for h in range(2):
                        u = 2 * it + h
                        ps = ppool.tile([128, MM_N], dt.float32)
                        for j in range(JT):
                            nc.tensor.matmul(
                                ps[32 * j : 32 * (j + 1), :],
                                basis_sb[:, :],
                                in_tile[:, h * BANK_COLS + MM_N * j
                                        : h * BANK_COLS + MM_N * (j + 1)],
                                start=True,
                                stop=True,
                                tile_position=(0, 32 * j),
                            )
                        nc.vector.tensor_copy(
                            out_tile[:, MM_N * u : MM_N * (u + 1)], ps[:, :]
                        )
                    # After the first half of the span's banks are cast,
                    # drain that half's rows early (finer out-DMAs overlap
                    # compute instead of bursting at the end).
                    if it == (U // 2 + 1) // 2 or it == HU - 1:
                        half = 0 if it == (U // 2 + 1) // 2 else 1
                        w0 = half * (U // 2) * MM_N
                        w1 = w0 + (U // 2) * MM_N
                        for j in range(JT):
                            for g in range(GROUPS):
                                row0 = 32 * j + 8 * g
                                nc.scalar.dma_start(
                                    out_t[span, half, j, g],
                                    out_tile[row0 : row0 + 8, w0:w1],
                                )
    return nc


def _dedupe_ldweights(m: dict) -> None:
    """Drop Ldweights instructions that reload the exact same stationary
    operand into the same PE array position as the previously retained one
    (the weights are static in this kernel).  Any waits on a dropped
    Ldweights migrate to the next instruction in the same engine stream."""
    def sig(ins):
        return json.dumps(
            {k: ins.get(k) for k in ("ins", "tile_position", "perf_mode",
                                     "is_transpose", "tile_size")},
            sort_keys=True,
        )

    def fix_block(b):
        last_by_pos = {}
        out = []
        pending_waits = []
        for ins in b.get("instructions", []):
            if ins.get("opcode") == "Ldweights":
                pos = tuple(ins.get("tile_position") or (0, 0))
                s = sig(ins)
                upd = (ins.get("sync_info") or {}).get("on_update", [])
                if last_by_pos.get(pos) == s and not upd:
                    pending_waits.extend(
                        (ins.get("sync_info") or {}).get("on_wait", []))
                    continue
                last_by_pos[pos] = s
            elif pending_waits and ins.get("engine") == "PE":
                si = ins.setdefault("sync_info", {"on_update": [], "on_wait": []})
                si["on_wait"] = pending_waits + si.get("on_wait", [])
                pending_waits = []
            out.append(ins)
        assert not pending_waits
        b["instructions"] = out
        for ch in b.get("blocks", []):
            fix_block(ch)

    for fn in m["functions"]:
        for b in fn.get("blocks", []):
            fix_block(b)


def _legalize_single_wait(bir_bytes: bytes) -> bytes:
    """Split multi-wait instructions: this walrus build's codegen accepts at
    most ONE sync-wait per ISA instruction.  Hoist all but the last wait onto
    NoOps inserted just before the instruction on the same engine stream
    (the sequencer executes them in order, so semantics are preserved)."""
    m = json.loads(bir_bytes)
    _dedupe_ldweights(m)
    n_split = 0

    def fix_block(b):
        nonlocal n_split
        out = []
        for ins in b.get("instructions", []):
            si = ins.get("sync_info")
            waits = (si or {}).get("on_wait", [])
            if len(waits) > 1 and ins.get("engine", "Unassigned") != "Unassigned":
                for w in waits[:-1]:
                    n_split += 1
                    out.append({
                        "debug": ins.get("debug", 0),
                        "engine": ins["engine"],
                        "ins": [],
                        "name": f"{ins['name']}-wsplit{n_split}",
                        "opcode": "NoOp",
                        "outs": [],
                        "sync_info": {"on_update": [], "on_wait": [w]},
                    })
                si["on_wait"] = [waits[-1]]
            out.append(ins)
        b["instructions"] = out
        for ch in b.get("blocks", []):
            fix_block(ch)

    for fn in m["functions"]:
        for b in fn.get("blocks", []):
            fix_block(b)
    return json.dumps(m).encode()


def _get_module():
    global _CACHED_NC
    if _CACHED_NC is None:
        nc = _build_module()
        orig = nc.to_json_bytes
        nc.to_json_bytes = lambda: _legalize_single_wait(orig())
        _CACHED_NC = nc
    return _CACHED_NC


def _host_basis(input_t: np.ndarray, model_stage) -> np.ndarray:
    """Packed stationary weights [120, 32] fp16: col 8g+jj = output jj of
    point-group g (jj 0-3 poly batch, 4-7 fourier batch)."""
    stage = int(model_stage)
    curr = min(stage, 3) if stage >= 0 else 3
    mask = np.zeros(KH, dtype=np.float64)
    for s, e, req in ((0, 3, 1), (3, 9, 2), (9, KH, 3)):
        if curr >= req:
            mask[s:e] = 1.0

    t = np.asarray(input_t, dtype=np.float64)
    Vp = np.stack([t ** i for i in range(4)], axis=0)           # [4, B]
    w = 2.0 * np.pi * np.arange(1, KH + 1, dtype=np.float64)    # [18]
    Cc = np.cos(np.outer(w, t)) * mask[:, None]                 # [18, B]
    Ss = np.sin(np.outer(w, t)) * mask[:, None]                 # [18, B]

    B8 = np.zeros((NCOEF, 8), dtype=np.float64)
    B8[0:4, 0:4] = Vp
    B8[4:22, 4:8] = Cc
    B8[22:40, 4:8] = Ss

    basis = np.zeros((GROUPS * NCOEF, 32), dtype=np.float64)
    for g in range(GROUPS):
        basis[NCOEF * g : NCOEF * (g + 1), 8 * g : 8 * g + 8] = B8
    return basis.astype(np.float16)


def kernel(input_t, poly_coeffs, fourier_a, fourier_b, model_stage):
    global LAST_RESULTS
    input_t = np.asarray(input_t, dtype=np.float32)
    poly_coeffs = np.asarray(poly_coeffs, dtype=np.float32)
    fourier_a = np.asarray(fourier_a, dtype=np.float32)
    fourier_b = np.asarray(fourier_b, dtype=np.float32)
    assert input_t.shape == (B,)
    assert poly_coeffs.shape == (N_POINTS, 4)
    assert fourier_a.shape == (N_POINTS, KH)
    assert fourier_b.shape == (N_POINTS, KH)

    basis = _host_basis(input_t, model_stage)

    # Pack per-core tables [120, C] fp16 with the device's column order:
    # table col (span*U + u)*BANK_COLS + j*MM_N + f  holds point
    # g*C + span*U*BANK_COLS + j*U*MM_N + u*MM_N + f   (j <-> u swapped so
    # each output row covers a contiguous DRAM run).
    W = np.concatenate([poly_coeffs, fourier_a, fourier_b], axis=1)
    W = W.astype(np.float16)                                    # [N, 40]
    Wp = np.zeros((N_CORES, NP, NCOEF), dtype=np.float16)
    Wp[:, :NC] = W.reshape(N_CORES, NC, NCOEF)
    Wp = Wp.reshape(N_CORES, GROUPS, SPANS, JT, U, MM_N, NCOEF)
    Wp = Wp.transpose(0, 1, 6, 2, 4, 3, 5)   # core, g, k, span, u, j, f
    tables = np.ascontiguousarray(Wp).reshape(N_CORES, GROUPS * NCOEF, C)
    # Tile the column axis so each in-DMA reads one contiguous DRAM block.
    NT = C // (2 * BANK_COLS)
    tables = np.ascontiguousarray(
        tables.reshape(N_CORES, GROUPS * NCOEF, NT, 2 * BANK_COLS)
        .transpose(0, 2, 1, 3))

    nc = _get_module()
    in_maps = [{"table": tables[c], "basis": basis} for c in range(N_CORES)]
    LAST_RESULTS = run_bass_kernel_spmd(nc, in_maps, core_ids=list(range(N_CORES)))
    results = LAST_RESULTS.results

    outs = []
    for r in results:
        ot = r["out_t"]  # [SPANS, 2, JT, GROUPS, 8, U*MM_N/2]
        o8 = ot.transpose(4, 3, 0, 2, 1, 5).reshape(8, NP)
        outs.append(o8[:, :NC].astype(np.float32))
    out = np.concatenate(outs, axis=1)
    return out[0:4], out[4:8]



# revision 3
# speedup vs baseline: 2.0902x; 2.0902x over previous
"""Trainium2 Bass kernel: batched polynomial + Fourier-series point evaluator.

Math: for each point n and each of B=4 times t_b:
    y_poly[b, n]    = sum_{i<4}  poly[n, i] * t_b^i
    y_fourier[b, n] = sum_{k<18} fa[n, k]*cos(w_k t_b) + fb[n, k]*sin(w_k t_b)
(with Fourier bands gated by model_stage).

Both outputs are one linear map of the 40 per-point coefficients:
Y[8, n] = Basis[40, 8].T @ W[n, :]  (basis from the 4 scalar times, host).

The tolerance budget (2e-2 rel L2) lets the coefficient tables be int8
(per-coefficient scales folded into the basis rows) and the outputs be
int8 (per-output 1/s_out folded into the basis cols) -> 40 B/point in,
8 B/point out, ~2.9x less HBM traffic than fp16 tables.

Per-core layout (points sharded 8 ways, NC = 2^18 = 32 banks x 8192):
  - coefficient split 32+8: "A" = coeffs 0..31, "B" = coeffs 32..39.
    One PSUM bank [128, 512] covers 8192 points: point (bank, strip j,
    group g, col f).  A-matmul per strip j: K=128 (4 groups x 32 A-coeffs)
    at tile_position (0, 32j); B-matmul: K=32 (4 groups x 8 B-coeffs) at
    tile_position (32j, 32j), accumulating.  All 128 contraction rows,
    all 128 PSUM partitions used; zero padding anywhere.
  - int8 tables DMA'd in 20 KB/partition blocks (~380 GB/s); DVE casts
    int8->fp16 at 2 elem/cyc/lane, ACT takes a share + drains PSUM to
    int8 (fp32->int8 rounds-to-nearest and saturates in HW).
  - psum value = y_j / s_out_j with |y| <= ~5.3 sigma < 127 by scale
    choice (s_out = 5.5 sigma / 127, sigma from exact column moments).
"""

import json

import numpy as np

import concourse.bass as bass
import concourse.mybir as mybir
import concourse.tile as tile
from concourse.bass_utils import run_bass_kernel_spmd

# Problem constants (hardcoded per harness contract).
B = 4
N_POINTS = 128 ** 3            # 2097152
N_CORES = 8
NC = N_POINTS // N_CORES       # 262144 points per core
KH = 18                        # harmonics
NCOEF = 40                     # 4 poly + 18 cos + 18 sin
KA, KB = 32, 8                 # coefficient split (A: K=128 matmul, B: K=32)

BANKS = 32                     # PSUM-bank fills per core (8192 points each)
MM_N = 512                     # matmul moving free size (one PSUM bank fp32)
BANK_COLS = 4 * MM_N + MM_N    # table cols per bank: A (4x512) + B (512)
NCHUNK = 4                     # in-DMA chunks per core
CHB = BANKS // NCHUNK          # banks per chunk (8)
CHC = CHB * BANK_COLS          # table cols per chunk (20480)
G4 = 4                         # banks per PSUM tile / drain group

IN_SIGMA = 4.3                 # int8 input grid clips at +-4.3 sigma
OUT_SIGMA = 5.1                # int8 output grid covers +-5.1 sigma

_CACHED_NC = None
LAST_RESULTS = None            # BassKernelResults of the most recent run


def _build_module():
    nc = bass.Bass()
    dt = mybir.dt

    table = nc.dram_tensor("table", [NCHUNK, 128, CHC], dt.int8,
                           kind="ExternalInput")
    basis_a = nc.dram_tensor("basis_a", [128, 32], dt.float16,
                             kind="ExternalInput")
    basis_b = nc.dram_tensor("basis_b", [128, 32], dt.float16,
                             kind="ExternalInput")
    out_t = nc.dram_tensor("out_t", [NCHUNK, 128, CHB * MM_N], dt.int8,
                           kind="ExternalOutput")

    with tile.TileContext(nc) as tc:
        with (
            tc.tile_pool(name="const", bufs=1) as cpool,
            tc.tile_pool(name="in8", bufs=2) as i8pool,
            tc.tile_pool(name="in16", bufs=2) as f16pool,
            tc.tile_pool(name="psum", bufs=2, space="PSUM") as ppool,
            tc.tile_pool(name="outp", bufs=1) as opool,
        ):
            ba = cpool.tile([128, 32], dt.float16)
            bb_t = cpool.tile([128, 32], dt.float16)
            nc.sync.dma_start(ba[:, :], basis_a[:, :])
            nc.sync.dma_start(bb_t[:, :], basis_b[:, :])

            out_tile = opool.tile([128, BANKS * MM_N], dt.int8)

            gidx = 0
            for ch in range(NCHUNK):
                it8 = i8pool.tile([128, CHC], dt.int8)
                nc.sync.dma_start(it8[:, :], table[ch])
                f16 = f16pool.tile([128, CHC], dt.float16)
                for g4 in range(CHB // G4):
                    c0 = g4 * G4 * BANK_COLS
                    # cast split: ACT takes 2 of the 8 groups, DVE the rest
                    if gidx in (2, 5):
                        nc.scalar.copy(f16[:, c0:c0 + G4 * BANK_COLS],
                                       it8[:, c0:c0 + G4 * BANK_COLS])
                    else:
                        nc.vector.tensor_copy(f16[:, c0:c0 + G4 * BANK_COLS],
                                              it8[:, c0:c0 + G4 * BANK_COLS])
                    ps = ppool.tile([128, G4 * MM_N], dt.float32)
                    for bb in range(G4):
                        coff = c0 + bb * BANK_COLS
                        pslice = ps[:, bb * MM_N:(bb + 1) * MM_N]
                        for j in range(4):
                            nc.tensor.matmul(
                                pslice[32 * j:32 * (j + 1), :],
                                ba[:, :],
                                f16[:, coff + MM_N * j:coff + MM_N * (j + 1)],
                                start=True, stop=False,
                                tile_position=(0, 32 * j),
                            )
                        for j in range(4):
                            nc.tensor.matmul(
                                pslice[32 * j:32 * (j + 1), :],
                                bb_t[32 * j:32 * (j + 1), :],
                                f16[32 * j:32 * (j + 1),
                                    coff + 4 * MM_N:coff + 5 * MM_N],
                                start=False, stop=True,
                                tile_position=(32 * j, 32 * j),
                            )
                    o0 = (ch * CHB + g4 * G4) * MM_N
                    # drain on the engine that did NOT cast this group
                    if gidx in (2, 5):
                        nc.vector.tensor_copy(
                            out_tile[:, o0:o0 + G4 * MM_N], ps[:, :])
                    else:
                        nc.scalar.copy(
                            out_tile[:, o0:o0 + G4 * MM_N], ps[:, :])
                    gidx += 1
                oc = ch * CHB * MM_N
                nc.scalar.dma_start(out_t[ch],
                                    out_tile[:, oc:oc + CHB * MM_N])
    return nc


def _dedupe_ldweights(m: dict) -> None:
    """Drop Ldweights whose full 32x32-cell coverage of the PE array already
    holds the exact same stationary data (tracked per cell, so loads at
    overlapping tile_positions correctly invalidate each other); migrate
    their waits."""
    def sig(ins):
        return json.dumps(
            {k: ins.get(k) for k in ("ins", "tile_position", "perf_mode",
                                     "is_transpose", "tile_size")},
            sort_keys=True,
        )

    def cells(ins):
        r0, c0 = tuple(ins.get("tile_position") or (0, 0))
        k, mm = tuple(ins.get("tile_size") or (128, 128))
        return [(r, c)
                for r in range(r0 // 32, (r0 + k + 31) // 32)
                for c in range(c0 // 32, (c0 + mm + 31) // 32)]

    def fix_block(b):
        cell_sig = {}
        out = []
        pending_waits = []
        for ins in b.get("instructions", []):
            if ins.get("opcode") == "Ldweights":
                s = sig(ins)
                cov = cells(ins)
                upd = (ins.get("sync_info") or {}).get("on_update", [])
                if all(cell_sig.get(c) == s for c in cov) and not upd:
                    pending_waits.extend(
                        (ins.get("sync_info") or {}).get("on_wait", []))
                    continue
                for c in cov:
                    cell_sig[c] = s
            elif pending_waits and ins.get("engine") == "PE":
                si = ins.setdefault("sync_info", {"on_update": [], "on_wait": []})
                si["on_wait"] = pending_waits + si.get("on_wait", [])
                pending_waits = []
            out.append(ins)
        assert not pending_waits
        b["instructions"] = out
        for ch in b.get("blocks", []):
            fix_block(ch)

    for fn in m["functions"]:
        for b in fn.get("blocks", []):
            fix_block(b)


def _legalize_single_wait(bir_bytes: bytes) -> bytes:
    """Split multi-wait instructions: this walrus build's codegen accepts at
    most ONE sync-wait per ISA instruction."""
    m = json.loads(bir_bytes)
    _dedupe_ldweights(m)
    n_split = 0

    def fix_block(b):
        nonlocal n_split
        out = []
        for ins in b.get("instructions", []):
            si = ins.get("sync_info")
            waits = (si or {}).get("on_wait", [])
            if len(waits) > 1 and ins.get("engine", "Unassigned") != "Unassigned":
                for w in waits[:-1]:
                    n_split += 1
                    out.append({
                        "debug": ins.get("debug", 0),
                        "engine": ins["engine"],
                        "ins": [],
                        "name": f"{ins['name']}-wsplit{n_split}",
                        "opcode": "NoOp",
                        "outs": [],
                        "sync_info": {"on_update": [], "on_wait": [w]},
                    })
                si["on_wait"] = [waits[-1]]
            out.append(ins)
        b["instructions"] = out
        for ch in b.get("blocks", []):
            fix_block(ch)

    for fn in m["functions"]:
        for b in fn.get("blocks", []):
            fix_block(b)
    return json.dumps(m).encode()


def _get_module():
    global _CACHED_NC
    if _CACHED_NC is None:
        nc = _build_module()
        orig = nc.to_json_bytes
        nc.to_json_bytes = lambda: _legalize_single_wait(orig())
        _CACHED_NC = nc
    return _CACHED_NC


def _host_basis(input_t: np.ndarray, model_stage) -> np.ndarray:
    """Dense [NCOEF, 8] fp64 basis: col jj<4 = poly output for t_jj,
    col 4+jj = fourier output for t_jj (band-masked by model_stage)."""
    stage = int(model_stage)
    curr = min(stage, 3) if stage >= 0 else 3
    mask = np.zeros(KH, dtype=np.float64)
    for s, e, req in ((0, 3, 1), (3, 9, 2), (9, KH, 3)):
        if curr >= req:
            mask[s:e] = 1.0

    t = np.asarray(input_t, dtype=np.float64)
    w = 2.0 * np.pi * np.arange(1, KH + 1, dtype=np.float64)
    Bas = np.zeros((NCOEF, 8), dtype=np.float64)
    for i in range(4):
        Bas[i, 0:4] = t ** i
    Bas[4:22, 4:8] = np.cos(np.outer(w, t)) * mask[:, None]
    Bas[22:40, 4:8] = np.sin(np.outer(w, t)) * mask[:, None]
    return Bas


def kernel(input_t, poly_coeffs, fourier_a, fourier_b, model_stage):
    global LAST_RESULTS
    input_t = np.asarray(input_t, dtype=np.float32)
    poly_coeffs = np.asarray(poly_coeffs, dtype=np.float32)
    fourier_a = np.asarray(fourier_a, dtype=np.float32)
    fourier_b = np.asarray(fourier_b, dtype=np.float32)
    assert input_t.shape == (B,)
    assert poly_coeffs.shape == (N_POINTS, 4)
    assert fourier_a.shape == (N_POINTS, KH)
    assert fourier_b.shape == (N_POINTS, KH)

    Bas = _host_basis(input_t, model_stage)                     # [40, 8]

    # --- input quantization: per-coefficient scales, clip at IN_SIGMA ---
    W = np.concatenate([poly_coeffs, fourier_a, fourier_b], axis=1)  # [N, 40]
    m2 = np.mean(W.astype(np.float64) ** 2, axis=0)             # [40]
    s_in = np.minimum(np.abs(W).astype(np.float64).max(axis=0),
                      IN_SIGMA * np.sqrt(m2)) / 127.0
    s_in[s_in == 0.0] = 1.0
    q = np.clip(np.rint(W / s_in.astype(np.float32)), -127, 127
                ).astype(np.int8)                               # [N, 40]
    sigma = np.sqrt((m2[:, None] * Bas ** 2).sum(axis=0))       # [8]
    s_out = OUT_SIGMA * sigma / 127.0
    s_out[s_out == 0.0] = 1.0

    # --- basis with scales folded: row k *= s_in[k], col j /= s_out[j] ---
    BasS = Bas * s_in[:, None] / s_out[None, :]                 # [40, 8]
    basis_a = np.zeros((128, 32), dtype=np.float16)
    basis_b = np.zeros((128, 32), dtype=np.float16)
    for g in range(4):
        basis_a[32 * g:32 * (g + 1), 8 * g:8 * (g + 1)] = BasS[:KA]
        for j in range(4):
            basis_b[32 * j + 8 * g:32 * j + 8 * (g + 1),
                    8 * g:8 * (g + 1)] = BasS[KA:]

    # --- pack per-core int8 tables ---
    # point p_local = bank*8192 + strip_j*2048 + group_g*512 + f
    qc = q.reshape(N_CORES, BANKS, 4, 4, MM_N, NCOEF)  # c, b, j, g, f, k
    qA = qc[..., :KA].transpose(0, 1, 3, 5, 2, 4)      # c, b, g, a, j, f
    qA = np.ascontiguousarray(qA).reshape(N_CORES, BANKS, 128, 4 * MM_N)
    qB = qc[..., KA:].transpose(0, 1, 2, 3, 5, 4)      # c, b, j, g, p, f
    qB = np.ascontiguousarray(qB).reshape(N_CORES, BANKS, 128, MM_N)
    tbl = np.concatenate([qA, qB], axis=3)             # c, b, 128, 2560
    tbl = tbl.reshape(N_CORES, NCHUNK, CHB, 128, BANK_COLS)
    tbl = np.ascontiguousarray(tbl.transpose(0, 1, 3, 2, 4)).reshape(
        N_CORES, NCHUNK, 128, CHC)

    nc = _get_module()
    in_maps = [{"table": tbl[c], "basis_a": basis_a, "basis_b": basis_b}
               for c in range(N_CORES)]
    LAST_RESULTS = run_bass_kernel_spmd(nc, in_maps, core_ids=list(range(N_CORES)))
    results = LAST_RESULTS.results

    outs = []
    s_out32 = s_out.astype(np.float32)
    for r in results:
        ot = r["out_t"]                       # [4, 128, 4096] int8
        o = ot.reshape(NCHUNK, 4, 4, 8, CHB, MM_N)  # ch, j, g, jj, bb, f
        o = o.transpose(3, 0, 4, 1, 2, 5).reshape(8, NC)  # jj, p_local
        outs.append(o)
    out = np.concatenate(outs, axis=1).astype(np.float32)  # [8, N]
    out *= s_out32[:, None]
    return out[0:4], out[4:8]


# revision 4
# speedup vs baseline: 2.5374x; 1.2139x over previous
"""Trainium2 Bass kernel: batched polynomial + Fourier-series point evaluator.

Math: for each point n and each of B=4 times t_b:
    y_poly[b, n]    = sum_{i<4}  poly[n, i] * t_b^i
    y_fourier[b, n] = sum_{k<18} fa[n, k]*cos(w_k t_b) + fb[n, k]*sin(w_k t_b)
(with Fourier bands gated by model_stage).

Both outputs are one linear map of the 40 per-point coefficients:
Y[8, n] = Basis[40, 8].T @ W[n, :]  (basis from the 4 scalar times, host).

The tolerance budget (2e-2 rel L2) lets the coefficient tables be int8
(per-coefficient scales folded into the basis rows) and the outputs be
int8 (per-output 1/s_out folded into the basis cols) -> 40 B/point in,
8 B/point out, ~2.9x less HBM traffic than fp16 tables.

Coefficient bytes are packed two-per-uint16 word (offset-128 encoding)
so the int8->fp16 expansion runs in the DVE's 4x perf mode (8-bit
sources cap at 2x).  Per word v = (hi+128)<<8 | (lo+128):
    u   = v AND 255          (uint16->uint16, 4x)
    lo  = u * 1   - 128      (uint16->fp16,   4x)  = q_lo exactly
    hi  = v / 256 - 128.5    (uint16->fp16,   4x)  = q_hi + q_lo/256
The q_lo/256 leak lands on a *different point's* coefficient (lo and hi
blocks cover different banks), i.e. mean-zero noise ~0.11 LSB rms -- it
just bumps the quantization noise a few percent.

Per-core layout (points sharded 8 ways, NC = 2^18 = 32 banks x 8192):
coefficient split 32+8: one PSUM bank [128, 512] covers 8192 points
(bank, strip j, group g, col f).  A-matmul per strip j: K=128 (4 groups
x 32 A-coeffs) at tile_position (0, 32j); B-matmul: K=32 at (32j, 32j),
accumulating.  All 128 contraction rows and PSUM partitions used, zero
padding.  PSUM value = y_j / s_out_j, |y| <= ~5.3 sigma < 127 by scale
choice; fp32->int8 drain rounds-to-nearest and saturates in HW.
"""

import json

import numpy as np

import concourse.bass as bass
import concourse.mybir as mybir
import concourse.tile as tile
from concourse.bass_utils import run_bass_kernel_spmd

# Problem constants (hardcoded per harness contract).
B = 4
N_POINTS = 128 ** 3            # 2097152
N_CORES = 8
NC = N_POINTS // N_CORES       # 262144 points per core
KH = 18                        # harmonics
NCOEF = 40                     # 4 poly + 18 cos + 18 sin
KA, KB = 32, 8                 # coefficient split (A: K=128 matmul, B: K=32)

BANKS = 32                     # PSUM-bank fills per core (8192 points each)
MM_N = 512                     # matmul moving free size (one PSUM bank fp32)
BANK_COLS = 4 * MM_N + MM_N    # fp16 cols per bank: A (4x512) + B (512)
G4 = 4                         # banks per group (one in-DMA / PSUM tile)
NGRP = BANKS // G4             # 8 groups per core
GRP_C = G4 * BANK_COLS         # 10240 fp16 cols per group
GRP_W = GRP_C // 2             # 5120 uint16 words per group
OCH = 2                        # groups per out-DMA

ACT_HI_GROUPS = (1, 4, 6)      # groups whose hi-op runs on ACT instead of DVE

IN_SIGMA = 4.3                 # int8 input grid clips at +-4.3 sigma
OUT_SIGMA = 5.1                # int8 output grid covers +-5.1 sigma

_CACHED_NC = None
LAST_RESULTS = None            # BassKernelResults of the most recent run


def _build_module():
    nc = bass.Bass()
    dt = mybir.dt
    ALU = mybir.AluOpType

    table = nc.dram_tensor("table", [NGRP, 128, GRP_W], dt.uint16,
                           kind="ExternalInput")
    basis_a = nc.dram_tensor("basis_a", [128, 32], dt.float16,
                             kind="ExternalInput")
    basis_b = nc.dram_tensor("basis_b", [128, 32], dt.float16,
                             kind="ExternalInput")
    out_t = nc.dram_tensor("out_t", [NGRP // OCH, 128, OCH * G4 * MM_N],
                           dt.int8, kind="ExternalOutput")

    with tile.TileContext(nc) as tc:
        with (
            tc.tile_pool(name="const", bufs=1) as cpool,
            tc.tile_pool(name="inw", bufs=3) as wpool,
            tc.tile_pool(name="andt", bufs=2) as apool,
            tc.tile_pool(name="in16", bufs=2) as fpool,
            tc.tile_pool(name="psum", bufs=2, space="PSUM") as ppool,
            tc.tile_pool(name="outp", bufs=1) as opool,
        ):
            ba = cpool.tile([128, 32], dt.float16)
            bb_t = cpool.tile([128, 32], dt.float16)
            out_tile = opool.tile([128, BANKS * MM_N], dt.int8)

            for g in range(NGRP):
                tw = wpool.tile([128, GRP_W], dt.uint16)
                nc.sync.dma_start(tw[:, :], table[g])
                if g == 0:
                    nc.sync.dma_start(ba[:, :], basis_a[:, :])
                    nc.sync.dma_start(bb_t[:, :], basis_b[:, :])
                ut = apool.tile([128, GRP_W], dt.uint16)
                f16 = fpool.tile([128, GRP_C], dt.float16)
                nc.vector.tensor_scalar(ut[:, :], tw[:, :], 255, None,
                                        ALU.bitwise_and)
                nc.vector.tensor_scalar(f16[:, 0:GRP_W], ut[:, :],
                                        1.0, 128.0, ALU.mult, ALU.subtract)
                if g in ACT_HI_GROUPS:
                    nc.scalar.activation(f16[:, GRP_W:GRP_C], tw[:, :],
                                         mybir.ActivationFunctionType.Copy,
                                         bias=-128.5, scale=1.0 / 256.0)
                else:
                    nc.vector.tensor_scalar(f16[:, GRP_W:GRP_C], tw[:, :],
                                            1.0 / 256.0, 128.5,
                                            ALU.mult, ALU.subtract)

                ps = ppool.tile([128, G4 * MM_N], dt.float32)
                for bb in range(G4):
                    coff = bb * BANK_COLS
                    pslice = ps[:, bb * MM_N:(bb + 1) * MM_N]
                    for j in range(4):
                        nc.tensor.matmul(
                            pslice[32 * j:32 * (j + 1), :],
                            ba[:, :],
                            f16[:, coff + MM_N * j:coff + MM_N * (j + 1)],
                            start=True, stop=False,
                            tile_position=(0, 32 * j),
                        )
                    for j in range(4):
                        nc.tensor.matmul(
                            pslice[32 * j:32 * (j + 1), :],
                            bb_t[32 * j:32 * (j + 1), :],
                            f16[32 * j:32 * (j + 1),
                                coff + 4 * MM_N:coff + 5 * MM_N],
                            start=False, stop=True,
                            tile_position=(32 * j, 32 * j),
                        )
                o0 = g * G4 * MM_N
                nc.scalar.copy(out_tile[:, o0:o0 + G4 * MM_N], ps[:, :])
                if g % OCH == OCH - 1:
                    oc = (g // OCH) * OCH * G4 * MM_N
                    nc.scalar.dma_start(
                        out_t[g // OCH],
                        out_tile[:, oc:oc + OCH * G4 * MM_N])
    return nc


def _dedupe_ldweights(m: dict) -> None:
    """Drop Ldweights whose full 32x32-cell coverage of the PE array already
    holds the exact same stationary data (tracked per cell, so loads at
    overlapping tile_positions correctly invalidate each other); migrate
    their waits."""
    def sig(ins):
        return json.dumps(
            {k: ins.get(k) for k in ("ins", "tile_position", "perf_mode",
                                     "is_transpose", "tile_size")},
            sort_keys=True,
        )

    def cells(ins):
        r0, c0 = tuple(ins.get("tile_position") or (0, 0))
        k, mm = tuple(ins.get("tile_size") or (128, 128))
        return [(r, c)
                for r in range(r0 // 32, (r0 + k + 31) // 32)
                for c in range(c0 // 32, (c0 + mm + 31) // 32)]

    def fix_block(b):
        cell_sig = {}
        out = []
        pending_waits = []
        for ins in b.get("instructions", []):
            if ins.get("opcode") == "Ldweights":
                s = sig(ins)
                cov = cells(ins)
                upd = (ins.get("sync_info") or {}).get("on_update", [])
                if all(cell_sig.get(c) == s for c in cov) and not upd:
                    pending_waits.extend(
                        (ins.get("sync_info") or {}).get("on_wait", []))
                    continue
                for c in cov:
                    cell_sig[c] = s
            elif pending_waits and ins.get("engine") == "PE":
                si = ins.setdefault("sync_info", {"on_update": [], "on_wait": []})
                si["on_wait"] = pending_waits + si.get("on_wait", [])
                pending_waits = []
            out.append(ins)
        assert not pending_waits
        b["instructions"] = out
        for ch in b.get("blocks", []):
            fix_block(ch)

    for fn in m["functions"]:
        for b in fn.get("blocks", []):
            fix_block(b)


def _legalize_single_wait(bir_bytes: bytes) -> bytes:
    """Split multi-wait instructions: this walrus build's codegen accepts at
    most ONE sync-wait per ISA instruction."""
    m = json.loads(bir_bytes)
    _dedupe_ldweights(m)
    n_split = 0

    def fix_block(b):
        nonlocal n_split
        out = []
        for ins in b.get("instructions", []):
            si = ins.get("sync_info")
            waits = (si or {}).get("on_wait", [])
            if len(waits) > 1 and ins.get("engine", "Unassigned") != "Unassigned":
                for w in waits[:-1]:
                    n_split += 1
                    out.append({
                        "debug": ins.get("debug", 0),
                        "engine": ins["engine"],
                        "ins": [],
                        "name": f"{ins['name']}-wsplit{n_split}",
                        "opcode": "NoOp",
                        "outs": [],
                        "sync_info": {"on_update": [], "on_wait": [w]},
                    })
                si["on_wait"] = [waits[-1]]
            out.append(ins)
        b["instructions"] = out
        for ch in b.get("blocks", []):
            fix_block(ch)

    for fn in m["functions"]:
        for b in fn.get("blocks", []):
            fix_block(b)
    return json.dumps(m).encode()


def _get_module():
    global _CACHED_NC
    if _CACHED_NC is None:
        nc = _build_module()
        orig = nc.to_json_bytes
        nc.to_json_bytes = lambda: _legalize_single_wait(orig())
        _CACHED_NC = nc
    return _CACHED_NC


def _host_basis(input_t: np.ndarray, model_stage) -> np.ndarray:
    """Dense [NCOEF, 8] fp64 basis: col jj<4 = poly output for t_jj,
    col 4+jj = fourier output for t_jj (band-masked by model_stage)."""
    stage = int(model_stage)
    curr = min(stage, 3) if stage >= 0 else 3
    mask = np.zeros(KH, dtype=np.float64)
    for s, e, req in ((0, 3, 1), (3, 9, 2), (9, KH, 3)):
        if curr >= req:
            mask[s:e] = 1.0

    t = np.asarray(input_t, dtype=np.float64)
    w = 2.0 * np.pi * np.arange(1, KH + 1, dtype=np.float64)
    Bas = np.zeros((NCOEF, 8), dtype=np.float64)
    for i in range(4):
        Bas[i, 0:4] = t ** i
    Bas[4:22, 4:8] = np.cos(np.outer(w, t)) * mask[:, None]
    Bas[22:40, 4:8] = np.sin(np.outer(w, t)) * mask[:, None]
    return Bas


def kernel(input_t, poly_coeffs, fourier_a, fourier_b, model_stage):
    global LAST_RESULTS
    input_t = np.asarray(input_t, dtype=np.float32)
    poly_coeffs = np.asarray(poly_coeffs, dtype=np.float32)
    fourier_a = np.asarray(fourier_a, dtype=np.float32)
    fourier_b = np.asarray(fourier_b, dtype=np.float32)
    assert input_t.shape == (B,)
    assert poly_coeffs.shape == (N_POINTS, 4)
    assert fourier_a.shape == (N_POINTS, KH)
    assert fourier_b.shape == (N_POINTS, KH)

    Bas = _host_basis(input_t, model_stage)                     # [40, 8]

    # --- input quantization: per-coefficient scales, clip at IN_SIGMA ---
    W = np.concatenate([poly_coeffs, fourier_a, fourier_b], axis=1)  # [N, 40]
    m2 = np.mean(W.astype(np.float64) ** 2, axis=0)             # [40]
    s_in = np.minimum(np.abs(W).astype(np.float64).max(axis=0),
                      IN_SIGMA * np.sqrt(m2)) / 127.0
    s_in[s_in == 0.0] = 1.0
    q = np.clip(np.rint(W / s_in.astype(np.float32)), -127, 127
                ).astype(np.int16)                              # [N, 40]

    # --- output scales from exact column moments ---
    sigma = np.sqrt((m2[:, None] * Bas ** 2).sum(axis=0))       # [8]
    s_out = OUT_SIGMA * sigma / 127.0
    s_out[s_out == 0.0] = 1.0

    # --- basis with scales folded: row k *= s_in[k], col j /= s_out[j] ---
    BasS = Bas * s_in[:, None] / s_out[None, :]                 # [40, 8]
    basis_a = np.zeros((128, 32), dtype=np.float16)
    basis_b = np.zeros((128, 32), dtype=np.float16)
    for g in range(4):
        basis_a[32 * g:32 * (g + 1), 8 * g:8 * (g + 1)] = BasS[:KA]
        for j in range(4):
            basis_b[32 * j + 8 * g:32 * j + 8 * (g + 1),
                    8 * g:8 * (g + 1)] = BasS[KA:]

    # --- pack per-core byte tables, then pair bytes into uint16 words ---
    # point p_local = bank*8192 + strip_j*2048 + group_g*512 + f
    qc = q.reshape(N_CORES, BANKS, 4, 4, MM_N, NCOEF)  # c, b, j, g, f, k
    qA = qc[..., :KA].transpose(0, 1, 3, 5, 2, 4)      # c, b, g, a, j, f
    qA = np.ascontiguousarray(qA).reshape(N_CORES, BANKS, 128, 4 * MM_N)
    qB = qc[..., KA:].transpose(0, 1, 2, 3, 5, 4)      # c, b, j, g, p, f
    qB = np.ascontiguousarray(qB).reshape(N_CORES, BANKS, 128, MM_N)
    byt = np.concatenate([qA, qB], axis=3)             # c, b, 128, 2560
    byt = byt.reshape(N_CORES, NGRP, G4, 128, BANK_COLS)
    byt = np.ascontiguousarray(byt.transpose(0, 1, 3, 2, 4)).reshape(
        N_CORES, NGRP, 128, GRP_C) + 128               # offset-128, int16
    # word i = hi-byte (col GRP_W+i) << 8 | lo-byte (col i)
    tbl = ((byt[..., GRP_W:].astype(np.uint16) << 8)
           | byt[..., :GRP_W].astype(np.uint16))       # c, NGRP, 128, GRP_W

    nc = _get_module()
    in_maps = [{"table": tbl[c], "basis_a": basis_a, "basis_b": basis_b}
               for c in range(N_CORES)]
    LAST_RESULTS = run_bass_kernel_spmd(nc, in_maps, core_ids=list(range(N_CORES)))
    results = LAST_RESULTS.results

    outs = []
    for r in results:
        ot = r["out_t"]                       # [4, 128, 4096] int8
        o = ot.reshape(NGRP // OCH, 4, 4, 8, OCH * G4, MM_N)
        o = o.transpose(3, 0, 4, 1, 2, 5).reshape(8, NC)  # jj, p_local
        outs.append(o)
    out = np.concatenate(outs, axis=1).astype(np.float32)  # [8, N]
    out *= s_out.astype(np.float32)[:, None]
    return out[0:4], out[4:8]
